# revision 7
# baseline (speedup 1.0000x reference)
"""Cross-attention kernel for Trainium2 (8 NeuronCores, SPMD).

Problem: B=4, Nq=1024, Nk=2048, D=512, 8 heads x 64 head-dim, fp32,
full-tensor bias added to scores before softmax.

Sharding: (batch, query-half) -> 8 disjoint shards, one per core. Each core
computes its own (512, 512) slice of the output; no collectives needed.
K/V projections are computed redundantly on the two cores sharing a batch.

Device layout: attention tensors kept transposed (feature/key dim on
partitions) so every matmul contraction lands on the partition axis:
  QT[d, q] = (SCALE*Wq) @ xT          KT[d, k] = Wk @ ctxT
  V[k, i]  = ctxT.T @ Wv.T
  ST[k, q] = KT_h.T @ QT_h            (contraction over the 64 head dims;
                                       the two heads of a pair sit in row
                                       groups 0-1/2-3 of the PE array and
                                       run concurrently)
  E = exp(ST) * exp(biasT - 4)        (ACT exp; DVE multiply against a
                                       step-0 broadcast of the host-side
                                       exp(bias - 4).T tile, so the bias
                                       add becomes a multiply and fp16
                                       cannot overflow)
  out2T[i(+1), q] = [V_h | 1].T @ E   (ones column yields softmax row-sums
                                       in the same accumulation)
  OT = out2T[0:64] * recip(sum)       (DVE reciprocal of the sum row, then
                                       a rank-2 selector matmul broadcasts
                                       the per-query factors across the 128
                                       partitions of the pair -- no DMA
                                       round trips on the critical path)
  yT[d, q] = Wo @ OT + bo             (bo enters as a rank-1 matmul that
                                       opens the PSUM accumulation; ACT
                                       evacuates to fp16 for the store)
Host transposes yT back. Matmul operands are fp16 (fp32 PSUM accumulate).
DMA schedule: ctx arrives in 512-column chunks in consumption order so the
first projection matmul issues ~3us in and the PE HAM warms early; eB
streams on the scalar queue; weights for the tail (wo, bo) load last.
K/Q projections for pair p+1 and V projections ride the attention loop as
TensorE gap filler (the exp chain is the attention-phase bottleneck).
"""

import numpy as np
import concourse.bass as bass
import concourse.bacc as bacc
import concourse.mybir as mybir
import concourse.tile as tile
from concourse import bass_utils

HEADS = 8
DH = 64
D = 512
NQ = 512          # queries per core (Nq=1024 split in halves)
NK = 2048
KC = NK // 128    # 16 key chunks
SCALE = DH ** -0.5
BSHIFT = 4.0      # exp(bias - BSHIFT): keeps fp16 weights in range

F32 = mybir.dt.float32
F16 = mybir.dt.float16
AF = mybir.ActivationFunctionType


def _bcast2(ap, n):
    """[128, F] -> [128, n, F] with a step-0 middle dim."""
    return bass.AP(ap.tensor, ap.offset, [ap.ap[0], [0, n], ap.ap[1]])


def _build_nc():
    nc = bacc.Bacc("TRN2", target_bir_lowering=False, debug=False)

    xT_d = nc.dram_tensor("xT", [D, NQ], F16, kind="ExternalInput")
    ctxT_d = nc.dram_tensor("ctxT", [D, NK], F16, kind="ExternalInput")
    expB_d = nc.dram_tensor("expB", [NK, NQ], F16, kind="ExternalInput")
    wqT_d = nc.dram_tensor("wqT", [D, D], F16, kind="ExternalInput")
    wkT_d = nc.dram_tensor("wkT", [D, D], F16, kind="ExternalInput")
    wvT_d = nc.dram_tensor("wvT", [D, D], F16, kind="ExternalInput")
    woT_d = nc.dram_tensor("woT", [D, D], F16, kind="ExternalInput")
    bo_d = nc.dram_tensor("bo", [1, D], F16, kind="ExternalInput")
    yT_d = nc.dram_tensor("yT", [D, NQ], F16, kind="ExternalOutput")

    with tile.TileContext(nc) as tc, nc.allow_low_precision(
            reason="fp16 matmul operands, fp32 accumulation"):
        with (
            tc.tile_pool(name="const", bufs=1) as const,
            tc.tile_pool(name="main", bufs=1) as main,
            tc.tile_pool(name="work", bufs=6) as work,
            tc.tile_pool(name="norm", bufs=3) as norm,
            tc.tile_pool(name="ctxp", bufs=1) as ctxp,
        ):
            # ---- loads: consumption order, three queues ----
            wq = [const.tile([128, D], F16, name=f"wq{i}", tag=f"wq{i}") for i in range(4)]
            wk = [const.tile([128, D], F16, name=f"wk{i}", tag=f"wk{i}") for i in range(4)]
            wv = [const.tile([128, D], F16, name=f"wv{i}", tag=f"wv{i}") for i in range(4)]
            wo = [const.tile([128, D], F16, name=f"wo{i}", tag=f"wo{i}") for i in range(4)]
            boro = const.tile([1, D], F16, name="boro", tag="boro")
            onesF = const.tile([128, 1], F32, name="onesF", tag="onesF")
            nc.vector.memset(onesF, 1.0)
            onesq = const.tile([1, NQ], F16, name="onesq", tag="onesq")
            nc.vector.tensor_copy(onesq, onesF[0:1, 0:1].broadcast_to([1, NQ]))
            # selector for broadcasting the two per-query norm factors of a
            # head pair across partitions 0-63 / 64-127 (rows 0 and 32 --
            # compute APs must start on a 32-aligned partition; the zero
            # rows in between contribute nothing)
            e2 = const.tile([33, 128], F16, name="e2", tag="e2")
            nc.vector.memset(e2, 0.0)
            nc.vector.memset(e2[0:1, 0:DH], 1.0)
            nc.vector.memset(e2[32:33, DH:128], 1.0)
            rr = const.tile([33, NQ], F16, name="rr", tag="rr")
            nc.vector.memset(rr, 0.0)

            ctx = [ctxp.tile([128, NK], F16, name=f"ctx{i}", tag=f"ctx{i}") for i in range(4)]
            xts = [ctxp.tile([128, NQ], F16, name=f"xts{i}", tag=f"xts{i}") for i in range(4)]
            # sync queue: wk, then ctx in 512-col chunks (nt-major so the
            # first k-proj group can start after ~1MB), then tail weights
            for i in range(4):
                nc.sync.dma_start(out=wk[i], in_=wkT_d[i * 128:(i + 1) * 128, :])
            for nt in range(4):
                nsl = slice(nt * 512, (nt + 1) * 512)
                for i in range(4):
                    nc.sync.dma_start(out=ctx[i][:, nsl],
                                      in_=ctxT_d[i * 128:(i + 1) * 128, nsl])
            # gpsimd queue: x + Wq (for the early Q proj), then Wv
            for i in range(4):
                sl = slice(i * 128, (i + 1) * 128)
                nc.gpsimd.dma_start(out=xts[i], in_=xT_d[sl, :])
                nc.gpsimd.dma_start(out=wq[i], in_=wqT_d[sl, :])
            for i in range(4):
                nc.gpsimd.dma_start(out=wv[i], in_=wvT_d[i * 128:(i + 1) * 128, :])

            KT = [main.tile([128, NK], F16, name=f"KT{i}", tag=f"KT{i}") for i in range(4)]
            QT = [main.tile([128, NQ], F16, name=f"QT{i}", tag=f"QT{i}") for i in range(4)]
            OT = [main.tile([128, NQ], F16, name=f"OT{i}", tag=f"OT{i}") for i in range(4)]
            Vo = [main.tile([128, HEADS, DH + 1], F16, name=f"Vo{c}", tag=f"Vo{c}")
                  for c in range(KC)]
            eB = [main.tile([128, NQ], F16, name=f"eB{c}", tag=f"eB{c}") for c in range(KC)]
            for c in range(KC):
                nc.vector.tensor_copy(
                    Vo[c][:, :, DH], onesF[:, 0:1].broadcast_to([128, HEADS]))
            # scalar queue: the full exp(bias) stream
            for c in range(KC):
                nc.scalar.dma_start(out=eB[c], in_=expB_d[c * 128:(c + 1) * 128, :])
            # tail-only weights, after the ctx stream
            for i in range(4):
                sl = slice(i * 128, (i + 1) * 128)
                nc.sync.dma_start(out=wo[i], in_=woT_d[sl, :])
            nc.sync.dma_start(out=boro, in_=bo_d[:, :])

            def k_proj_group(psA, mi, nt):
                msl = slice(mi * 128, (mi + 1) * 128)
                nsl = slice(nt * 512, (nt + 1) * 512)
                ps = psA.tile([128, 512], F32, name="proj", tag="proj")
                for ki in range(4):
                    nc.tensor.matmul(
                        ps, wk[ki][:, msl], ctx[ki][:, nsl],
                        start=(ki == 0), stop=(ki == 3))
                nc.vector.tensor_copy(KT[mi][:, nsl], ps)

            def v_proj_group(psA, c):
                csl = slice(c * 128, (c + 1) * 128)
                ps = psA.tile([128, 512], F32, name="vproj", tag="proj")
                for ki in range(4):
                    nc.tensor.matmul(
                        ps, ctx[ki][:, csl], wv[ki],
                        start=(ki == 0), stop=(ki == 3))
                nc.vector.tensor_copy(
                    Vo[c][:, :, 0:DH],
                    ps.rearrange("p (h d) -> p h d", h=HEADS))

            def q_proj_group(psA, mi):
                msl = slice(mi * 128, (mi + 1) * 128)
                ps = psA.tile([128, 512], F32, name="proj", tag="proj")
                for ki in range(4):
                    nc.tensor.matmul(
                        ps, wq[ki][:, msl], xts[ki],
                        start=(ki == 0), stop=(ki == 3))
                nc.vector.tensor_copy(QT[mi], ps)

            # ---- upfront: just enough to start pair-0 attention ----
            with tc.tile_pool(name="psA0", bufs=3, space="PSUM") as psA0:
                k_proj_group(psA0, 0, 0)
                k_proj_group(psA0, 0, 1)
                q_proj_group(psA0, 0)
                for c in range(2):
                    v_proj_group(psA0, c)

            # ---- attention (head pairs) with interleaved projections ----
            with (
                tc.tile_pool(name="psS", bufs=2, space="PSUM") as psS,
                tc.tile_pool(name="psO", bufs=2, space="PSUM") as psO,
                tc.tile_pool(name="psA", bufs=1, space="PSUM") as psA,
                tc.tile_pool(name="psR", bufs=1, space="PSUM") as psR,
            ):
                for hp in range(4):
                    h0, h1 = 2 * hp, 2 * hp + 1
                    lo, hi = slice(0, DH), slice(DH, 128)
                    o2a = psO.tile([DH + 1, NQ], F32, name="o2a", tag="o2")
                    o2b = psO.tile([DH + 1, NQ], F32, name="o2b", tag="o2")
                    for c in range(KC):
                        csl = slice(c * 128, (c + 1) * 128)
                        s = psS.tile([128, 2, NQ], F32, name="s", tag="s")
                        nc.tensor.matmul(
                            s[:, 0, :], KT[hp][lo, csl], QT[hp][lo, :],
                            start=True, stop=True)
                        nc.tensor.matmul(
                            s[:, 1, :], KT[hp][hi, csl], QT[hp][hi, :],
                            start=True, stop=True)
                        e1 = work.tile([128, 2, NQ], F16, name="e1", tag="e1")
                        nc.scalar.activation(e1, s, AF.Exp)
                        et = work.tile([128, 2, NQ], F16, name="et", tag="et")
                        nc.vector.tensor_mul(et, e1, _bcast2(eB[c][:, :], 2))
                        nc.tensor.matmul(
                            o2a, Vo[c][:, h0, :], et[:, 0, :],
                            start=(c == 0), stop=(c == KC - 1))
                        nc.tensor.matmul(
                            o2b, Vo[c][:, h1, :], et[:, 1, :],
                            start=(c == 0), stop=(c == KC - 1))
                        # TensorE filler: V groups + the remaining K groups
                        # of this pair + next pair's K/Q
                        if hp == 0:
                            if c == 0:
                                k_proj_group(psA, 0, 2)
                            elif c == 1:
                                k_proj_group(psA, 0, 3)
                            if c <= 13:
                                v_proj_group(psA, c + 2)
                            if c == 14:
                                k_proj_group(psA, 1, 0)
                            elif c == 15:
                                q_proj_group(psA, 1)
                        else:
                            if c in (0, 2, 4):
                                k_proj_group(psA, hp, c // 2 + 1)
                            elif hp < 3:
                                if c == 12:
                                    k_proj_group(psA, hp + 1, 0)
                                elif c == 13:
                                    q_proj_group(psA, hp + 1)
                    # normalize: reciprocal of the sum rows, broadcast the
                    # per-query factors across the pair's 128 partitions
                    # with a rank-2 selector matmul, one fused multiply
                    oUp = norm.tile([128, NQ], F16, name="oUp", tag="oUp")
                    nc.vector.tensor_copy(oUp[0:DH, :], o2a[0:DH, :])
                    nc.vector.tensor_copy(oUp[DH:128, :], o2b[0:DH, :])
                    nc.vector.reciprocal(rr[0:1, :], o2a[DH:DH + 1, :])
                    nc.vector.reciprocal(rr[32:33, :], o2b[DH:DH + 1, :])
                    rb = psR.tile([128, NQ], F32, name="rb", tag="rb")
                    nc.tensor.matmul(rb, e2, rr, start=True, stop=True)
                    nc.vector.tensor_mul(OT[hp], oUp, rb)

            # ---- output projection + bias (ki-outer: the ki<3 partial
            # sums run while the last head pair is still normalizing;
            # bo enters as a rank-1 matmul opening the accumulation) ----
            with tc.tile_pool(name="psY", bufs=1, space="PSUM") as psY:
                pss = [psY.tile([128, NQ], F32, name=f"yTp{mi}", tag=f"yTp{mi}")
                       for mi in range(4)]
                for mi in range(4):
                    msl = slice(mi * 128, (mi + 1) * 128)
                    nc.tensor.matmul(
                        pss[mi], boro[:, msl], onesq,
                        start=True, stop=False)
                for ki in range(4):
                    for mi in range(4):
                        msl = slice(mi * 128, (mi + 1) * 128)
                        nc.tensor.matmul(
                            pss[mi], wo[ki][:, msl], OT[ki],
                            start=False, stop=(ki == 3))
                for mi in range(4):
                    msl = slice(mi * 128, (mi + 1) * 128)
                    ysb = norm.tile([128, NQ], F16, name="ysb", tag="ysb")
                    nc.scalar.copy(ysb, pss[mi])
                    if mi % 2 == 0:
                        nc.sync.dma_start(out=yT_d[msl, :], in_=ysb)
                    else:
                        nc.gpsimd.dma_start(out=yT_d[msl, :], in_=ysb)

    nc.compile()
    return nc


_NC_CACHE = {}


def _get_nc():
    if "nc" not in _NC_CACHE:
        _NC_CACHE["nc"] = _build_nc()
    return _NC_CACHE["nc"]


def make_in_maps(x, context, bias, Wq, Wk, Wv, Wo, bo):
    x = np.asarray(x, dtype=np.float32)
    context = np.asarray(context, dtype=np.float32)
    bias = np.asarray(bias, dtype=np.float32)
    wqT = np.ascontiguousarray((np.asarray(Wq) * SCALE).T).astype(np.float16)
    wkT = np.ascontiguousarray(np.asarray(Wk).T).astype(np.float16)
    wvT = np.ascontiguousarray(np.asarray(Wv).T).astype(np.float16)
    woT = np.ascontiguousarray(np.asarray(Wo).T).astype(np.float16)
    bo2 = np.ascontiguousarray(np.asarray(bo).reshape(1, D)).astype(np.float16)

    in_maps = []
    for core in range(8):
        b, half = core // 2, core % 2
        qs = half * NQ
        in_maps.append({
            "xT": np.ascontiguousarray(x[b, qs:qs + NQ, :].T).astype(np.float16),
            "ctxT": np.ascontiguousarray(context[b].T).astype(np.float16),
            "expB": np.ascontiguousarray(
                np.exp(bias[b, qs:qs + NQ, :] - BSHIFT).T).astype(np.float16),
            "wqT": wqT, "wkT": wkT, "wvT": wvT, "woT": woT, "bo": bo2,
        })
    return in_maps


def kernel(x, context, bias, Wq, Wk, Wv, Wo, bo):
    nc = _get_nc()
    in_maps = make_in_maps(x, context, bias, Wq, Wk, Wv, Wo, bo)
    res = bass_utils.run_bass_kernel_spmd(
        nc, in_maps, core_ids=list(range(8)), trace=False)

    out = np.empty((4, 2 * NQ, D), dtype=np.float32)
    for core in range(8):
        b, half = core // 2, core % 2
        qs = half * NQ
        out[b, qs:qs + NQ, :] = res.results[core]["yT"].astype(np.float32).T
    return out


# revision 10
# speedup vs baseline: 1.0340x; 1.0340x over previous
"""Cross-attention kernel for Trainium2 (8 NeuronCores, SPMD).

Problem: B=4, Nq=1024, Nk=2048, D=512, 8 heads x 64 head-dim, fp32,
full-tensor bias added to scores before softmax.

Sharding: (batch, query-half) -> 8 disjoint shards, one per core. Each core
computes its own (512, 512) slice of the output; no collectives needed.
K/V projections are computed redundantly on the two cores sharing a batch.

Device layout: attention tensors kept transposed (feature/key dim on
partitions) so every matmul contraction lands on the partition axis:
  QT[d, q] = (SCALE*Wq) @ xT          KT[d, k] = Wk @ ctxT
  V[k, i]  = ctxT.T @ Wv.T
  ST[k, q] = KT_h.T @ QT_h            (contraction over the 64 head dims;
                                       the two heads of a pair sit in row
                                       groups 0-1/2-3 of the PE array and
                                       run concurrently)
  E = exp(ST) * exp(biasT - 4)        (ACT exp; DVE multiply against a
                                       step-0 broadcast of the host-side
                                       exp(bias - 4).T tile, so the bias
                                       add becomes a multiply and fp16
                                       cannot overflow)
  out2T[i(+1), q] = [V_h | 1].T @ E   (ones column yields softmax row-sums
                                       in the same accumulation)
  OT = out2T[0:64] * recip(sum)       (DVE reciprocal of the sum row, then
                                       a rank-2 selector matmul broadcasts
                                       the per-query factors across the 128
                                       partitions of the pair -- no DMA
                                       round trips on the critical path)
  yT[d, q] = Wo @ OT + bo             (bo enters as a rank-1 matmul that
                                       opens the PSUM accumulation; ACT
                                       evacuates to fp16 for the store)
Host transposes yT back. Matmul operands are fp16 (fp32 PSUM accumulate).
DMA schedule: ctx arrives in 512-column chunks in consumption order so the
first projection matmul issues ~3us in and the PE HAM warms early; eB
streams on the scalar queue; weights for the tail (wo, bo) load last.
K/Q projections for pair p+1 and V projections ride the attention loop as
TensorE gap filler (the exp chain is the attention-phase bottleneck).
"""

import numpy as np
import concourse.bass as bass
import concourse.bacc as bacc
import concourse.mybir as mybir
import concourse.tile as tile
from concourse import bass_utils

HEADS = 8
DH = 64
D = 512
NQ = 512          # queries per core (Nq=1024 split in halves)
NK = 2048
KC = NK // 128    # 16 key chunks
SCALE = DH ** -0.5
BSHIFT = 4.0      # exp(bias - BSHIFT): keeps fp16 weights in range

F32 = mybir.dt.float32
F16 = mybir.dt.float16
AF = mybir.ActivationFunctionType


def _bcast2(ap, n):
    """[128, F] -> [128, n, F] with a step-0 middle dim."""
    return bass.AP(ap.tensor, ap.offset, [ap.ap[0], [0, n], ap.ap[1]])


def _build_nc():
    nc = bacc.Bacc("TRN2", target_bir_lowering=False, debug=False)

    xT_d = nc.dram_tensor("xT", [D, NQ], F16, kind="ExternalInput")
    ctxT_d = nc.dram_tensor("ctxT", [D, NK], F16, kind="ExternalInput")
    expB_d = nc.dram_tensor("expB", [NK, NQ], F16, kind="ExternalInput")
    wqT_d = nc.dram_tensor("wqT", [D, D], F16, kind="ExternalInput")
    wkT_d = nc.dram_tensor("wkT", [D, D], F16, kind="ExternalInput")
    wvT_d = nc.dram_tensor("wvT", [D, D], F16, kind="ExternalInput")
    woT_d = nc.dram_tensor("woT", [D, D], F16, kind="ExternalInput")
    bo_d = nc.dram_tensor("bo", [1, D], F16, kind="ExternalInput")
    yT_d = nc.dram_tensor("yT", [D, NQ], F16, kind="ExternalOutput")

    with tile.TileContext(nc) as tc, nc.allow_low_precision(
            reason="fp16 matmul operands, fp32 accumulation"):
        with (
            tc.tile_pool(name="const", bufs=1) as const,
            tc.tile_pool(name="main", bufs=1) as main,
            tc.tile_pool(name="work", bufs=6) as work,
            tc.tile_pool(name="norm", bufs=3) as norm,
            tc.tile_pool(name="ctxp", bufs=1) as ctxp,
        ):
            # ---- loads: consumption order, three queues ----
            wq = [const.tile([128, D], F16, name=f"wq{i}", tag=f"wq{i}") for i in range(4)]
            wk = [const.tile([128, D], F16, name=f"wk{i}", tag=f"wk{i}") for i in range(4)]
            wv = [const.tile([128, D], F16, name=f"wv{i}", tag=f"wv{i}") for i in range(4)]
            wo = [const.tile([128, D], F16, name=f"wo{i}", tag=f"wo{i}") for i in range(4)]
            boro = const.tile([1, D], F16, name="boro", tag="boro")
            onesF = const.tile([128, 1], F32, name="onesF", tag="onesF")
            nc.vector.memset(onesF, 1.0)
            onesq = const.tile([1, NQ], F16, name="onesq", tag="onesq")
            nc.vector.tensor_copy(onesq, onesF[0:1, 0:1].broadcast_to([1, NQ]))
            # selector for broadcasting the two per-query norm factors of a
            # head pair across partitions 0-63 / 64-127 (rows 0 and 32 --
            # compute APs must start on a 32-aligned partition; the zero
            # rows in between contribute nothing)
            e2 = const.tile([33, 128], F16, name="e2", tag="e2")
            nc.vector.memset(e2, 0.0)
            nc.vector.memset(e2[0:1, 0:DH], 1.0)
            nc.vector.memset(e2[32:33, DH:128], 1.0)
            ss2 = const.tile([33, NQ], F16, name="ss2", tag="ss2")
            nc.vector.memset(ss2, 0.0)

            ctx = [ctxp.tile([128, NK], F16, name=f"ctx{i}", tag=f"ctx{i}") for i in range(4)]
            xts = [ctxp.tile([128, NQ], F16, name=f"xts{i}", tag=f"xts{i}") for i in range(4)]
            # sync queue: wk, then ctx in 512-col chunks (nt-major so the
            # first k-proj group can start after ~1MB), then tail weights
            for i in range(4):
                nc.sync.dma_start(out=wk[i], in_=wkT_d[i * 128:(i + 1) * 128, :])
            for nt in range(2):
                nsl = slice(nt * 1024, (nt + 1) * 1024)
                for i in range(4):
                    nc.sync.dma_start(out=ctx[i][:, nsl],
                                      in_=ctxT_d[i * 128:(i + 1) * 128, nsl])
            # gpsimd queue: x + Wq (for the early Q proj), then Wv
            for i in range(4):
                sl = slice(i * 128, (i + 1) * 128)
                nc.gpsimd.dma_start(out=xts[i], in_=xT_d[sl, :])
                nc.gpsimd.dma_start(out=wq[i], in_=wqT_d[sl, :])
            for i in range(4):
                nc.gpsimd.dma_start(out=wv[i], in_=wvT_d[i * 128:(i + 1) * 128, :])

            KT = [main.tile([128, NK], F16, name=f"KT{i}", tag=f"KT{i}") for i in range(4)]
            QT = [main.tile([128, NQ], F16, name=f"QT{i}", tag=f"QT{i}") for i in range(4)]
            OT = [main.tile([128, NQ], F16, name=f"OT{i}", tag=f"OT{i}") for i in range(4)]
            Vo = [main.tile([128, HEADS, DH + 1], F16, name=f"Vo{c}", tag=f"Vo{c}")
                  for c in range(KC)]
            eB = [main.tile([128, NQ], F16, name=f"eB{c}", tag=f"eB{c}") for c in range(KC)]
            for c in range(KC):
                nc.vector.tensor_copy(
                    Vo[c][:, :, DH], onesF[:, 0:1].broadcast_to([128, HEADS]))
            # scalar queue: the full exp(bias) stream
            for c in range(KC):
                nc.scalar.dma_start(out=eB[c], in_=expB_d[c * 128:(c + 1) * 128, :])
            # tail-only weights, after the ctx stream
            for i in range(4):
                sl = slice(i * 128, (i + 1) * 128)
                nc.sync.dma_start(out=wo[i], in_=woT_d[sl, :])
            nc.sync.dma_start(out=boro, in_=bo_d[:, :])

            def k_proj_group(psA, mi, nt):
                msl = slice(mi * 128, (mi + 1) * 128)
                nsl = slice(nt * 512, (nt + 1) * 512)
                ps = psA.tile([128, 512], F32, name="proj", tag="proj")
                for ki in range(4):
                    nc.tensor.matmul(
                        ps, wk[ki][:, msl], ctx[ki][:, nsl],
                        start=(ki == 0), stop=(ki == 3))
                nc.vector.tensor_copy(KT[mi][:, nsl], ps)

            def v_proj_group(psA, c):
                csl = slice(c * 128, (c + 1) * 128)
                ps = psA.tile([128, 512], F32, name="vproj", tag="proj")
                for ki in range(4):
                    nc.tensor.matmul(
                        ps, ctx[ki][:, csl], wv[ki],
                        start=(ki == 0), stop=(ki == 3))
                nc.vector.tensor_copy(
                    Vo[c][:, :, 0:DH],
                    ps.rearrange("p (h d) -> p h d", h=HEADS))

            def q_proj_group(psA, mi):
                msl = slice(mi * 128, (mi + 1) * 128)
                ps = psA.tile([128, 512], F32, name="proj", tag="proj")
                for ki in range(4):
                    nc.tensor.matmul(
                        ps, wq[ki][:, msl], xts[ki],
                        start=(ki == 0), stop=(ki == 3))
                nc.vector.tensor_copy(QT[mi], ps)

            # ---- upfront: just enough to start pair-0 attention ----
            with tc.tile_pool(name="psA0", bufs=3, space="PSUM") as psA0:
                k_proj_group(psA0, 0, 0)
                k_proj_group(psA0, 0, 1)
                q_proj_group(psA0, 0)
                for c in range(2):
                    v_proj_group(psA0, c)

            # ---- attention (head pairs) with interleaved projections ----
            with (
                tc.tile_pool(name="psS", bufs=2, space="PSUM") as psS,
                tc.tile_pool(name="psO", bufs=2, space="PSUM") as psO,
                tc.tile_pool(name="psA", bufs=1, space="PSUM") as psA,
                tc.tile_pool(name="psR", bufs=1, space="PSUM") as psR,
            ):
                for hp in range(4):
                    h0, h1 = 2 * hp, 2 * hp + 1
                    lo, hi = slice(0, DH), slice(DH, 128)
                    o2a = psO.tile([DH + 1, NQ], F32, name="o2a", tag="o2")
                    o2b = psO.tile([DH + 1, NQ], F32, name="o2b", tag="o2")
                    for c in range(KC):
                        csl = slice(c * 128, (c + 1) * 128)
                        s = psS.tile([128, 2, NQ], F32, name="s", tag="s")
                        nc.tensor.matmul(
                            s[:, 0, :], KT[hp][lo, csl], QT[hp][lo, :],
                            start=True, stop=True)
                        nc.tensor.matmul(
                            s[:, 1, :], KT[hp][hi, csl], QT[hp][hi, :],
                            start=True, stop=True)
                        e1 = work.tile([128, 2, NQ], F16, name="e1", tag="e1")
                        nc.scalar.activation(e1, s, AF.Exp)
                        et = work.tile([128, 2, NQ], F16, name="et", tag="et")
                        nc.vector.tensor_mul(et, e1, _bcast2(eB[c][:, :], 2))
                        nc.tensor.matmul(
                            o2a, Vo[c][:, h0, :], et[:, 0, :],
                            start=(c == 0), stop=(c == KC - 1))
                        nc.tensor.matmul(
                            o2b, Vo[c][:, h1, :], et[:, 1, :],
                            start=(c == 0), stop=(c == KC - 1))
                        # TensorE filler: V groups + the remaining K groups
                        # of this pair + next pair's K/Q
                        if hp == 0:
                            if c == 0:
                                k_proj_group(psA, 0, 2)
                            elif c == 1:
                                k_proj_group(psA, 0, 3)
                            if c <= 13:
                                v_proj_group(psA, c + 2)
                            if c == 14:
                                k_proj_group(psA, 1, 0)
                            elif c == 15:
                                q_proj_group(psA, 1)
                        else:
                            if c in (0, 2, 4):
                                k_proj_group(psA, hp, c // 2 + 1)
                            elif hp < 3:
                                if c == 12:
                                    k_proj_group(psA, hp + 1, 0)
                                elif c == 13:
                                    q_proj_group(psA, hp + 1)
                    # normalize: reciprocal of the sum rows, broadcast the
                    # per-query factors across the pair's 128 partitions
                    # with a rank-2 selector matmul, one fused multiply
                    oUp = norm.tile([128, NQ], F16, name="oUp", tag="oUp")
                    nc.vector.tensor_copy(oUp[0:DH, :], o2a[0:DH, :])
                    nc.vector.tensor_copy(oUp[DH:128, :], o2b[0:DH, :])
                    nc.vector.tensor_copy(ss2[0:1, :], o2a[DH:DH + 1, :])
                    nc.vector.tensor_copy(ss2[32:33, :], o2b[DH:DH + 1, :])
                    rb = psR.tile([128, NQ], F32, name="rb", tag="rb")
                    nc.tensor.matmul(rb, e2, ss2, start=True, stop=True)
                    rrb = norm.tile([128, NQ], F32, name="rrb", tag="rrb")
                    nc.vector.reciprocal_approx_fast(out=rrb, in_=rb)
                    nc.vector.tensor_mul(OT[hp], oUp, rrb)

            # ---- output projection + bias (ki-outer: the ki<3 partial
            # sums run while the last head pair is still normalizing;
            # bo enters as a rank-1 matmul opening the accumulation) ----
            with tc.tile_pool(name="psY", bufs=1, space="PSUM") as psY:
                pss = [psY.tile([128, NQ], F32, name=f"yTp{mi}", tag=f"yTp{mi}")
                       for mi in range(4)]
                for mi in range(4):
                    msl = slice(mi * 128, (mi + 1) * 128)
                    nc.tensor.matmul(
                        pss[mi], boro[:, msl], onesq,
                        start=True, stop=False)
                for ki in range(4):
                    for mi in range(4):
                        msl = slice(mi * 128, (mi + 1) * 128)
                        nc.tensor.matmul(
                            pss[mi], wo[ki][:, msl], OT[ki],
                            start=False, stop=(ki == 3))
                for mi in range(4):
                    msl = slice(mi * 128, (mi + 1) * 128)
                    ysb = norm.tile([128, NQ], F16, name="ysb", tag="ysb")
                    nc.scalar.copy(ysb, pss[mi])
                    if mi % 2 == 0:
                        nc.sync.dma_start(out=yT_d[msl, :], in_=ysb)
                    else:
                        nc.gpsimd.dma_start(out=yT_d[msl, :], in_=ysb)

    nc.compile()
    return nc


_NC_CACHE = {}


def _get_nc():
    if "nc" not in _NC_CACHE:
        _NC_CACHE["nc"] = _build_nc()
    return _NC_CACHE["nc"]


def make_in_maps(x, context, bias, Wq, Wk, Wv, Wo, bo):
    x = np.asarray(x, dtype=np.float32)
    context = np.asarray(context, dtype=np.float32)
    bias = np.asarray(bias, dtype=np.float32)
    wqT = np.ascontiguousarray((np.asarray(Wq) * SCALE).T).astype(np.float16)
    wkT = np.ascontiguousarray(np.asarray(Wk).T).astype(np.float16)
    wvT = np.ascontiguousarray(np.asarray(Wv).T).astype(np.float16)
    woT = np.ascontiguousarray(np.asarray(Wo).T).astype(np.float16)
    bo2 = np.ascontiguousarray(np.asarray(bo).reshape(1, D)).astype(np.float16)

    in_maps = []
    for core in range(8):
        b, half = core // 2, core % 2
        qs = half * NQ
        in_maps.append({
            "xT": np.ascontiguousarray(x[b, qs:qs + NQ, :].T).astype(np.float16),
            "ctxT": np.ascontiguousarray(context[b].T).astype(np.float16),
            "expB": np.ascontiguousarray(
                np.exp(bias[b, qs:qs + NQ, :] - BSHIFT).T).astype(np.float16),
            "wqT": wqT, "wkT": wkT, "wvT": wvT, "woT": woT, "bo": bo2,
        })
    return in_maps


def kernel(x, context, bias, Wq, Wk, Wv, Wo, bo):
    nc = _get_nc()
    in_maps = make_in_maps(x, context, bias, Wq, Wk, Wv, Wo, bo)
    res = bass_utils.run_bass_kernel_spmd(
        nc, in_maps, core_ids=list(range(8)), trace=False)

    out = np.empty((4, 2 * NQ, D), dtype=np.float32)
    for core in range(8):
        b, half = core // 2, core % 2
        qs = half * NQ
        out[b, qs:qs + NQ, :] = res.results[core]["yT"].astype(np.float32).T
    return out


# revision 16
# speedup vs baseline: 1.2022x; 1.1627x over previous
"""Cross-attention kernel for Trainium2 (8 NeuronCores, SPMD).

Problem: B=4, Nq=1024, Nk=2048, D=512, 8 heads x 64 head-dim, fp32,
full-tensor bias added to scores before softmax.

Sharding: (batch, query-half) -> 8 disjoint shards, one per core. Each core
computes its own (512, 512) slice of the output; no collectives needed.
K/V projections are computed redundantly on the two cores sharing a batch.

Device layout: attention tensors kept transposed (feature/key dim on
partitions) so every matmul contraction lands on the partition axis:
  QT[d, q] = (SCALE*Wq) @ xT          KT[d, k] = Wk @ ctxT
  V[k, i]  = ctxT.T @ Wv.T
  ST[k, q] = KT_h.T @ QT_h            (contraction over the 64 head dims;
                                       the two heads of a pair sit in row
                                       groups 0-1/2-3 of the PE array and
                                       run concurrently)
  E = exp(ST) * exp(biasT - 4)        (ACT exp; DVE multiply against a
                                       step-0 broadcast of the host-side
                                       exp(bias - 4).T tile, so the bias
                                       add becomes a multiply and fp16
                                       cannot overflow)
  out2T[i(+1), q] = [V_h | 1].T @ E   (ones column yields softmax row-sums
                                       in the same accumulation)
  OT = out2T[0:64] * recip(sum)       (DVE reciprocal of the sum row, then
                                       a rank-2 selector matmul broadcasts
                                       the per-query factors across the 128
                                       partitions of the pair -- no DMA
                                       round trips on the critical path)
  yT[d, q] = Wo @ OT + bo             (bo enters as a rank-1 matmul that
                                       opens the PSUM accumulation; ACT
                                       evacuates to fp16 for the store)
Host transposes yT back. Matmul operands are fp16 (fp32 PSUM accumulate).
DMA schedule: ctx arrives in 512-column chunks in consumption order so the
first projection matmul issues ~3us in and the PE HAM warms early; eB
streams on the scalar queue; weights for the tail (wo, bo) load last.
K/Q projections for pair p+1 and V projections ride the attention loop as
TensorE gap filler (the exp chain is the attention-phase bottleneck).
"""

import numpy as np
import concourse.bass as bass
import concourse.bacc as bacc
import concourse.mybir as mybir
import concourse.tile as tile
from concourse import bass_utils

HEADS = 8
DH = 64
D = 512
NQ = 512          # queries per core (Nq=1024 split in halves)
NK = 2048
KC = NK // 128    # 16 key chunks
SCALE = DH ** -0.5
BSHIFT = 4.0      # exp(bias - BSHIFT): keeps fp16 weights in range

F32 = mybir.dt.float32
F16 = mybir.dt.float16
AF = mybir.ActivationFunctionType


def _bcast2(ap, n):
    """[128, F] -> [128, n, F] with a step-0 middle dim."""
    return bass.AP(ap.tensor, ap.offset, [ap.ap[0], [0, n], ap.ap[1]])


def _build_nc():
    nc = bacc.Bacc("TRN2", target_bir_lowering=False, debug=False)

    xT_d = nc.dram_tensor("xT", [D, NQ], F16, kind="ExternalInput")
    ctxT_d = nc.dram_tensor("ctxT", [D, NK], F16, kind="ExternalInput")
    expB_d = nc.dram_tensor("expB", [NK, NQ], F16, kind="ExternalInput")
    wqT_d = nc.dram_tensor("wqT", [D, D], F16, kind="ExternalInput")
    wkT_d = nc.dram_tensor("wkT", [D, D], F16, kind="ExternalInput")
    wvT_d = nc.dram_tensor("wvT", [D, D], F16, kind="ExternalInput")
    woT_d = nc.dram_tensor("woT", [D, D], F16, kind="ExternalInput")
    bo_d = nc.dram_tensor("bo", [1, D], F16, kind="ExternalInput")
    yT_d = nc.dram_tensor("yT", [D, NQ], F16, kind="ExternalOutput")

    with tile.TileContext(nc) as tc, nc.allow_low_precision(
            reason="fp16 matmul operands, fp32 accumulation"):
        with (
            tc.tile_pool(name="const", bufs=1) as const,
            tc.tile_pool(name="main", bufs=1) as main,
            tc.tile_pool(name="work", bufs=6) as work,
            tc.tile_pool(name="norm", bufs=3) as norm,
            tc.tile_pool(name="ctxp", bufs=1) as ctxp,
        ):
            # ---- loads: consumption order, three queues ----
            wq = [const.tile([128, D], F16, name=f"wq{i}", tag=f"wq{i}") for i in range(4)]
            wk = [const.tile([128, D], F16, name=f"wk{i}", tag=f"wk{i}") for i in range(4)]
            wv = [const.tile([128, D], F16, name=f"wv{i}", tag=f"wv{i}") for i in range(4)]
            wo = [const.tile([128, D], F16, name=f"wo{i}", tag=f"wo{i}") for i in range(4)]
            boro = const.tile([1, D], F16, name="boro", tag="boro")
            onesF = const.tile([128, 1], F32, name="onesF", tag="onesF")
            nc.vector.memset(onesF, 1.0)
            onesq = const.tile([1, NQ], F16, name="onesq", tag="onesq")
            nc.vector.tensor_copy(onesq, onesF[0:1, 0:1].broadcast_to([1, NQ]))
            # selector for broadcasting the two per-query norm factors of a
            # head pair across partitions 0-63 / 64-127 (rows 0 and 32 --
            # compute APs must start on a 32-aligned partition; the zero
            # rows in between contribute nothing)
            e2 = const.tile([33, 128], F16, name="e2", tag="e2")
            nc.vector.memset(e2, 0.0)
            nc.vector.memset(e2[0:1, 0:DH], 1.0)
            nc.vector.memset(e2[32:33, DH:128], 1.0)
            ss2 = const.tile([33, NQ], F16, name="ss2", tag="ss2")
            nc.vector.memset(ss2, 0.0)

            ctx = [ctxp.tile([128, NK], F16, name=f"ctx{i}", tag=f"ctx{i}") for i in range(4)]
            xts = [ctxp.tile([128, NQ], F16, name=f"xts{i}", tag=f"xts{i}") for i in range(4)]
            # sync queue: wk, then ctx in 512-col chunks (nt-major so the
            # first k-proj group can start after ~1MB), then tail weights
            for i in range(4):
                nc.sync.dma_start(out=wk[i], in_=wkT_d[i * 128:(i + 1) * 128, :])
            for nt in range(2):
                nsl = slice(nt * 1024, (nt + 1) * 1024)
                for i in range(4):
                    nc.sync.dma_start(out=ctx[i][:, nsl],
                                      in_=ctxT_d[i * 128:(i + 1) * 128, nsl])
            # gpsimd queue: x + Wq (for the early Q proj), then Wv
            for i in range(4):
                sl = slice(i * 128, (i + 1) * 128)
                nc.gpsimd.dma_start(out=xts[i], in_=xT_d[sl, :])
                nc.gpsimd.dma_start(out=wq[i], in_=wqT_d[sl, :])
            for i in range(4):
                nc.gpsimd.dma_start(out=wv[i], in_=wvT_d[i * 128:(i + 1) * 128, :])

            KT = [main.tile([128, NK], F16, name=f"KT{i}", tag=f"KT{i}") for i in range(4)]
            QT = [main.tile([128, NQ], F16, name=f"QT{i}", tag=f"QT{i}") for i in range(4)]
            OT = [main.tile([128, NQ], F16, name=f"OT{i}", tag=f"OT{i}") for i in range(4)]
            Vo = [main.tile([128, HEADS, DH + 1], F16, name=f"Vo{c}", tag=f"Vo{c}")
                  for c in range(KC)]
            eB = [main.tile([128, NQ], F16, name=f"eB{c}", tag=f"eB{c}") for c in range(KC)]
            for c in range(KC):
                nc.vector.tensor_copy(
                    Vo[c][:, :, DH], onesF[:, 0:1].broadcast_to([128, HEADS]))
            # exp(bias) stream rides the sync queue after ctx -- the scalar
            # queue must stay free for the ACTIVATEs
            for c in range(KC):
                nc.sync.dma_start(out=eB[c], in_=expB_d[c * 128:(c + 1) * 128, :])
            # tail-only weights, after the ctx stream
            for i in range(4):
                sl = slice(i * 128, (i + 1) * 128)
                nc.sync.dma_start(out=wo[i], in_=woT_d[sl, :])
            nc.sync.dma_start(out=boro, in_=bo_d[:, :])

            def k_proj_group(psA, mi, nt):
                msl = slice(mi * 128, (mi + 1) * 128)
                nsl = slice(nt * 512, (nt + 1) * 512)
                ps = psA.tile([128, 512], F32, name="proj", tag="proj")
                for ki in range(4):
                    nc.tensor.matmul(
                        ps, wk[ki][:, msl], ctx[ki][:, nsl],
                        start=(ki == 0), stop=(ki == 3))
                nc.vector.tensor_copy(KT[mi][:, nsl], ps)

            def v_proj_group(psA, c, on_act=False):
                csl = slice(c * 128, (c + 1) * 128)
                ps = psA.tile([128, 512], F32, name="vproj", tag="proj")
                for ki in range(4):
                    nc.tensor.matmul(
                        ps, ctx[ki][:, csl], wv[ki],
                        start=(ki == 0), stop=(ki == 3))
                src = ps.rearrange("p (h d) -> p h d", h=HEADS)
                if on_act:
                    nc.scalar.copy(Vo[c][:, :, 0:DH], src)
                else:
                    nc.vector.tensor_copy(Vo[c][:, :, 0:DH], src)

            def q_proj_group(psA, mi):
                msl = slice(mi * 128, (mi + 1) * 128)
                ps = psA.tile([128, 512], F32, name="proj", tag="proj")
                for ki in range(4):
                    nc.tensor.matmul(
                        ps, wq[ki][:, msl], xts[ki],
                        start=(ki == 0), stop=(ki == 3))
                nc.vector.tensor_copy(QT[mi], ps)

            # ---- upfront: just enough to start pair-0 attention (the
            # later k groups are DMA-paced by the second ctx half) ----
            with tc.tile_pool(name="psA0", bufs=3, space="PSUM") as psA0:
                k_proj_group(psA0, 0, 0)
                k_proj_group(psA0, 0, 1)
                q_proj_group(psA0, 0)
                for c in range(2):
                    v_proj_group(psA0, c)
                k_proj_group(psA0, 0, 2)
                k_proj_group(psA0, 0, 3)

            # ---- attention (head pairs) with interleaved projections ----
            with (
                tc.tile_pool(name="psS", bufs=2, space="PSUM") as psS,
                tc.tile_pool(name="psO", bufs=2, space="PSUM") as psO,
                tc.tile_pool(name="psA", bufs=2, space="PSUM") as psA,
            ):
                for hp in range(4):
                    h0, h1 = 2 * hp, 2 * hp + 1
                    lo, hi = slice(0, DH), slice(DH, 128)
                    o2a = psO.tile([DH + 1, NQ], F32, name="o2a", tag="o2")
                    o2b = psO.tile([DH + 1, NQ], F32, name="o2b", tag="o2")
                    for c in range(KC):
                        csl = slice(c * 128, (c + 1) * 128)
                        s = psS.tile([128, 2, NQ], F32, name="s", tag="s")
                        nc.tensor.matmul(
                            s[:, 0, :], KT[hp][lo, csl], QT[hp][lo, :],
                            start=True, stop=True)
                        nc.tensor.matmul(
                            s[:, 1, :], KT[hp][hi, csl], QT[hp][hi, :],
                            start=True, stop=True)
                        e1 = work.tile([128, 2, NQ], F16, name="e1", tag="e1")
                        nc.scalar.activation(e1, s, AF.Exp)
                        et = work.tile([128, 2, NQ], F16, name="et", tag="et")
                        nc.vector.tensor_mul(et, e1, _bcast2(eB[c][:, :], 2))
                        nc.tensor.matmul(
                            o2a, Vo[c][:, h0, :], et[:, 0, :],
                            start=(c == 0), stop=(c == KC - 1))
                        nc.tensor.matmul(
                            o2b, Vo[c][:, h1, :], et[:, 1, :],
                            start=(c == 0), stop=(c == KC - 1))
                        # TensorE filler: V groups + the remaining K groups
                        # of this pair + next pair's K/Q
                        if hp == 0:
                            if c <= 13:
                                v_proj_group(psA, c + 2, on_act=(c % 3 == 2))
                            if c == 14:
                                k_proj_group(psA, 1, 0)
                            elif c == 15:
                                q_proj_group(psA, 1)
                        else:
                            if c in (0, 2, 4):
                                k_proj_group(psA, hp, c // 2 + 1)
                            elif hp < 3:
                                if c == 12:
                                    k_proj_group(psA, hp + 1, 0)
                                elif c == 13:
                                    q_proj_group(psA, hp + 1)
                    # normalize: reciprocal of the sum rows, broadcast the
                    # per-query factors across the pair's 128 partitions
                    # with a rank-2 selector matmul, one fused multiply
                    oUp = norm.tile([128, NQ], F16, name="oUp", tag="oUp")
                    nc.vector.tensor_copy(oUp[0:DH, :], o2a[0:DH, :])
                    nc.vector.tensor_copy(oUp[DH:128, :], o2b[0:DH, :])
                    nc.vector.tensor_copy(ss2[0:1, :], o2a[DH:DH + 1, :])
                    nc.vector.tensor_copy(ss2[32:33, :], o2b[DH:DH + 1, :])
                    rb = psA.tile([128, NQ], F32, name="rb", tag="proj")
                    nc.tensor.matmul(rb, e2, ss2, start=True, stop=True)
                    rrb = norm.tile([128, NQ], F32, name="rrb", tag="rrb")
                    nc.vector.reciprocal_approx_fast(out=rrb, in_=rb)
                    nc.vector.tensor_mul(OT[hp], oUp, rrb)

            # ---- output projection + bias (ki-outer: the ki<3 partial
            # sums run while the last head pair is still normalizing;
            # bo enters as a rank-1 matmul opening the accumulation) ----
            with tc.tile_pool(name="psY", bufs=1, space="PSUM") as psY:
                pss = [psY.tile([128, NQ], F32, name=f"yTp{mi}", tag=f"yTp{mi}")
                       for mi in range(4)]
                for mi in range(4):
                    msl = slice(mi * 128, (mi + 1) * 128)
                    nc.tensor.matmul(
                        pss[mi], boro[:, msl], onesq,
                        start=True, stop=False)
                for ki in range(4):
                    for mi in range(4):
                        msl = slice(mi * 128, (mi + 1) * 128)
                        nc.tensor.matmul(
                            pss[mi], wo[ki][:, msl], OT[ki],
                            start=False, stop=(ki == 3))
                for mi in range(4):
                    msl = slice(mi * 128, (mi + 1) * 128)
                    ysb = norm.tile([128, NQ], F16, name="ysb", tag="ysb")
                    nc.scalar.copy(ysb, pss[mi])
                    if mi % 2 == 0:
                        nc.sync.dma_start(out=yT_d[msl, :], in_=ysb)
                    else:
                        nc.gpsimd.dma_start(out=yT_d[msl, :], in_=ysb)

    nc.compile()
    return nc


_NC_CACHE = {}


def _get_nc():
    if "nc" not in _NC_CACHE:
        _NC_CACHE["nc"] = _build_nc()
    return _NC_CACHE["nc"]


def make_in_maps(x, context, bias, Wq, Wk, Wv, Wo, bo):
    x = np.asarray(x, dtype=np.float32)
    context = np.asarray(context, dtype=np.float32)
    bias = np.asarray(bias, dtype=np.float32)
    wqT = np.ascontiguousarray((np.asarray(Wq) * SCALE).T).astype(np.float16)
    wkT = np.ascontiguousarray(np.asarray(Wk).T).astype(np.float16)
    wvT = np.ascontiguousarray(np.asarray(Wv).T).astype(np.float16)
    woT = np.ascontiguousarray(np.asarray(Wo).T).astype(np.float16)
    bo2 = np.ascontiguousarray(np.asarray(bo).reshape(1, D)).astype(np.float16)

    in_maps = []
    for core in range(8):
        b, half = core // 2, core % 2
        qs = half * NQ
        in_maps.append({
            "xT": np.ascontiguousarray(x[b, qs:qs + NQ, :].T).astype(np.float16),
            "ctxT": np.ascontiguousarray(context[b].T).astype(np.float16),
            "expB": np.ascontiguousarray(
                np.exp(bias[b, qs:qs + NQ, :] - BSHIFT).T).astype(np.float16),
            "wqT": wqT, "wkT": wkT, "wvT": wvT, "woT": woT, "bo": bo2,
        })
    return in_maps


def kernel(x, context, bias, Wq, Wk, Wv, Wo, bo):
    nc = _get_nc()
    in_maps = make_in_maps(x, context, bias, Wq, Wk, Wv, Wo, bo)
    res = bass_utils.run_bass_kernel_spmd(
        nc, in_maps, core_ids=list(range(8)), trace=False)

    out = np.empty((4, 2 * NQ, D), dtype=np.float32)
    for core in range(8):
        b, half = core // 2, core % 2
        qs = half * NQ
        out[b, qs:qs + NQ, :] = res.results[core]["yT"].astype(np.float32).T
    return out


# revision 18
# speedup vs baseline: 1.2694x; 1.0559x over previous
"""Cross-attention kernel for Trainium2 (8 NeuronCores, SPMD).

Problem: B=4, Nq=1024, Nk=2048, D=512, 8 heads x 64 head-dim, fp32,
full-tensor bias added to scores before softmax.

Sharding: (batch, query-half) -> 8 disjoint shards, one per core. Each core
computes its own (512, 512) slice of the output; no collectives needed.
K/V projections are computed redundantly on the two cores sharing a batch.

Device layout: attention tensors kept transposed (feature/key dim on
partitions) so every matmul contraction lands on the partition axis:
  QT[d, q] = (SCALE*Wq) @ xT          KT[d, k] = Wk @ ctxT
  V[k, i]  = ctxT.T @ Wv.T
  ST[k, q] = KT_h.T @ QT_h            (contraction over the 64 head dims;
                                       the two heads of a pair sit in row
                                       groups 0-1/2-3 of the PE array and
                                       run concurrently)
  E = exp(ST) * exp(biasT - 4)        (ACT exp; DVE multiply against a
                                       step-0 broadcast of the host-side
                                       exp(bias - 4).T tile, so the bias
                                       add becomes a multiply and fp16
                                       cannot overflow)
  out2T[i(+1), q] = [V_h | 1].T @ E   (ones column yields softmax row-sums
                                       in the same accumulation)
  OT = out2T[0:64] * recip(sum)       (DVE reciprocal of the sum row, then
                                       a rank-2 selector matmul broadcasts
                                       the per-query factors across the 128
                                       partitions of the pair -- no DMA
                                       round trips on the critical path)
  yT[d, q] = Wo @ OT + bo             (bo enters as a rank-1 matmul that
                                       opens the PSUM accumulation; ACT
                                       evacuates to fp16 for the store)
Host transposes yT back. Matmul operands are fp16 (fp32 PSUM accumulate).
DMA schedule: ctx arrives in 512-column chunks in consumption order so the
first projection matmul issues ~3us in and the PE HAM warms early; eB
streams on the scalar queue; weights for the tail (wo, bo) load last.
K/Q projections for pair p+1 and V projections ride the attention loop as
TensorE gap filler (the exp chain is the attention-phase bottleneck).
"""

import numpy as np
import concourse.bass as bass
import concourse.bacc as bacc
import concourse.mybir as mybir
import concourse.tile as tile
from concourse import bass_utils

HEADS = 8
DH = 64
D = 512
NQ = 512          # queries per core (Nq=1024 split in halves)
NK = 2048
KC = NK // 128    # 16 key chunks
SCALE = DH ** -0.5
BSHIFT = 4.0      # exp(bias - BSHIFT): keeps fp16 weights in range

F32 = mybir.dt.float32
F16 = mybir.dt.float16
AF = mybir.ActivationFunctionType


def _bcast2(ap, n):
    """[128, F] -> [128, n, F] with a step-0 middle dim."""
    return bass.AP(ap.tensor, ap.offset, [ap.ap[0], [0, n], ap.ap[1]])


def _build_nc():
    nc = bacc.Bacc("TRN2", target_bir_lowering=False, debug=False)

    xT_d = nc.dram_tensor("xT", [D, NQ], F16, kind="ExternalInput")
    ctxT_d = nc.dram_tensor("ctxT", [D, NK], F16, kind="ExternalInput")
    expB_d = nc.dram_tensor("expB", [NK, NQ], F16, kind="ExternalInput")
    wqT_d = nc.dram_tensor("wqT", [D, D], F16, kind="ExternalInput")
    wkT_d = nc.dram_tensor("wkT", [D, D], F16, kind="ExternalInput")
    wvT_d = nc.dram_tensor("wvT", [D, D], F16, kind="ExternalInput")
    woT_d = nc.dram_tensor("woT", [D, D], F16, kind="ExternalInput")
    bo_d = nc.dram_tensor("bo", [1, D], F16, kind="ExternalInput")
    yT_d = nc.dram_tensor("yT", [D, NQ], F16, kind="ExternalOutput")

    with tile.TileContext(nc) as tc, nc.allow_low_precision(
            reason="fp16 matmul operands, fp32 accumulation"):
        with (
            tc.tile_pool(name="const", bufs=1) as const,
            tc.tile_pool(name="main", bufs=1) as main,
            tc.tile_pool(name="work", bufs=6) as work,
            tc.tile_pool(name="norm", bufs=3) as norm,
            tc.tile_pool(name="ctxp", bufs=1) as ctxp,
        ):
            # ---- loads: consumption order, three queues ----
            wq = [const.tile([128, D], F16, name=f"wq{i}", tag=f"wq{i}") for i in range(4)]
            wk = [const.tile([128, D], F16, name=f"wk{i}", tag=f"wk{i}") for i in range(4)]
            wv = [const.tile([128, D], F16, name=f"wv{i}", tag=f"wv{i}") for i in range(4)]
            wo = [const.tile([128, D], F16, name=f"wo{i}", tag=f"wo{i}") for i in range(4)]
            boro = const.tile([1, D], F16, name="boro", tag="boro")
            onesF = const.tile([128, 1], F32, name="onesF", tag="onesF")
            nc.vector.memset(onesF, 1.0)
            onesq = const.tile([1, NQ], F16, name="onesq", tag="onesq")
            nc.vector.tensor_copy(onesq, onesF[0:1, 0:1].broadcast_to([1, NQ]))
            # selector for broadcasting the two per-query norm factors of a
            # head pair across partitions 0-63 / 64-127 (rows 0 and 32 --
            # compute APs must start on a 32-aligned partition; the zero
            # rows in between contribute nothing)
            e2 = const.tile([33, 128], F16, name="e2", tag="e2")
            nc.vector.memset(e2, 0.0)
            nc.vector.memset(e2[0:1, 0:DH], 1.0)
            nc.vector.memset(e2[32:33, DH:128], 1.0)
            ss2 = const.tile([33, NQ], F16, name="ss2", tag="ss2")
            nc.vector.memset(ss2, 0.0)

            ctx = [ctxp.tile([128, NK], F16, name=f"ctx{i}", tag=f"ctx{i}") for i in range(4)]
            xts = [ctxp.tile([128, NQ], F16, name=f"xts{i}", tag=f"xts{i}") for i in range(4)]
            # sync queue: wk, then ctx in 512-col chunks (nt-major so the
            # first k-proj group can start after ~1MB), then tail weights
            for i in range(4):
                nc.sync.dma_start(out=wk[i], in_=wkT_d[i * 128:(i + 1) * 128, :])
            for nt in range(2):
                nsl = slice(nt * 1024, (nt + 1) * 1024)
                for i in range(4):
                    nc.sync.dma_start(out=ctx[i][:, nsl],
                                      in_=ctxT_d[i * 128:(i + 1) * 128, nsl])
            # gpsimd queue: x + Wq (for the early Q proj), then Wv
            for i in range(4):
                sl = slice(i * 128, (i + 1) * 128)
                nc.gpsimd.dma_start(out=xts[i], in_=xT_d[sl, :])
                nc.gpsimd.dma_start(out=wq[i], in_=wqT_d[sl, :])
            for i in range(4):
                nc.gpsimd.dma_start(out=wv[i], in_=wvT_d[i * 128:(i + 1) * 128, :])

            KT = [main.tile([128, NK], F16, name=f"KT{i}", tag=f"KT{i}") for i in range(4)]
            QT = [main.tile([128, NQ], F16, name=f"QT{i}", tag=f"QT{i}") for i in range(4)]
            OT = [main.tile([128, NQ], F16, name=f"OT{i}", tag=f"OT{i}") for i in range(4)]
            Vo = [main.tile([128, HEADS, DH + 1], F16, name=f"Vo{c}", tag=f"Vo{c}")
                  for c in range(KC)]
            eB = [main.tile([128, NQ], F16, name=f"eB{c}", tag=f"eB{c}") for c in range(KC)]
            for c in range(KC):
                nc.vector.tensor_copy(
                    Vo[c][:, :, DH], onesF[:, 0:1].broadcast_to([128, HEADS]))
            # exp(bias) stream rides the sync queue after ctx -- the scalar
            # queue must stay free for the ACTIVATEs
            for c in range(KC):
                nc.sync.dma_start(out=eB[c], in_=expB_d[c * 128:(c + 1) * 128, :])
            # tail-only weights, after the ctx stream
            for i in range(4):
                sl = slice(i * 128, (i + 1) * 128)
                nc.sync.dma_start(out=wo[i], in_=woT_d[sl, :])
            nc.sync.dma_start(out=boro, in_=bo_d[:, :])

            def k_proj_group(psA, mi, nt):
                msl = slice(mi * 128, (mi + 1) * 128)
                nsl = slice(nt * 512, (nt + 1) * 512)
                ps = psA.tile([128, 512], F32, name="proj", tag="proj")
                for ki in range(4):
                    nc.tensor.matmul(
                        ps, wk[ki][:, msl], ctx[ki][:, nsl],
                        start=(ki == 0), stop=(ki == 3))
                nc.vector.tensor_copy(KT[mi][:, nsl], ps)

            def v_proj_group(psA, c, on_act=False):
                csl = slice(c * 128, (c + 1) * 128)
                ps = psA.tile([128, 512], F32, name="vproj", tag="proj")
                for ki in range(4):
                    nc.tensor.matmul(
                        ps, ctx[ki][:, csl], wv[ki],
                        start=(ki == 0), stop=(ki == 3))
                src = ps.rearrange("p (h d) -> p h d", h=HEADS)
                if on_act:
                    nc.scalar.copy(Vo[c][:, :, 0:DH], src)
                else:
                    nc.vector.tensor_copy(Vo[c][:, :, 0:DH], src)

            def q_proj_group(psA, mi):
                msl = slice(mi * 128, (mi + 1) * 128)
                ps = psA.tile([128, 512], F32, name="proj", tag="proj")
                for ki in range(4):
                    nc.tensor.matmul(
                        ps, wq[ki][:, msl], xts[ki],
                        start=(ki == 0), stop=(ki == 3))
                nc.vector.tensor_copy(QT[mi], ps)

            # ---- upfront: just enough to start pair-0 attention (the
            # later k groups are DMA-paced by the second ctx half) ----
            with tc.tile_pool(name="psA0", bufs=3, space="PSUM") as psA0:
                k_proj_group(psA0, 0, 0)
                k_proj_group(psA0, 0, 1)
                q_proj_group(psA0, 0)
                for c in range(2):
                    v_proj_group(psA0, c)

            # ---- attention (head pairs) with interleaved projections ----
            with (
                tc.tile_pool(name="psS", bufs=2, space="PSUM") as psS,
                tc.tile_pool(name="psO", bufs=2, space="PSUM") as psO,
                tc.tile_pool(name="psA", bufs=2, space="PSUM") as psA,
            ):
                lo, hi = slice(0, DH), slice(DH, 128)

                def fillers(hp, c):
                    # pair 0 owns the V stream and its own late K groups;
                    # every pair prefetches the whole next pair's K/Q
                    if hp == 0:
                        if c == 0:
                            k_proj_group(psA, 0, 2)
                        elif c == 1:
                            k_proj_group(psA, 0, 3)
                        if c <= 13:
                            v_proj_group(psA, c + 2, on_act=(c % 3 == 2))
                    if hp < 3:
                        if c == 6:
                            k_proj_group(psA, hp + 1, 0)
                        elif c == 8:
                            k_proj_group(psA, hp + 1, 1)
                        elif c == 10:
                            k_proj_group(psA, hp + 1, 2)
                        elif c == 11:
                            q_proj_group(psA, hp + 1)
                        elif c == 13:
                            k_proj_group(psA, hp + 1, 3)

                # software-pipelined: scores run two chunks ahead of the
                # exp/mul/AV stage so each pair's first scores are already
                # in PSUM when the previous pair's exps drain
                s_tiles, o2t = {}, {}
                for g in range(64 + 2):
                    if g < 64:
                        hp, c = divmod(g, KC)
                        csl = slice(c * 128, (c + 1) * 128)
                        s = psS.tile([128, 2, NQ], F32, name="s", tag="s")
                        s_tiles[g] = s
                        nc.tensor.matmul(
                            s[:, 0, :], KT[hp][lo, csl], QT[hp][lo, :],
                            start=True, stop=True)
                        nc.tensor.matmul(
                            s[:, 1, :], KT[hp][hi, csl], QT[hp][hi, :],
                            start=True, stop=True)
                    if g >= 2:
                        hp, c = divmod(g - 2, KC)
                        h0, h1 = 2 * hp, 2 * hp + 1
                        if c == 0:
                            o2t[hp] = (
                                psO.tile([DH + 1, NQ], F32, name="o2a", tag="o2"),
                                psO.tile([DH + 1, NQ], F32, name="o2b", tag="o2"))
                        o2a, o2b = o2t[hp]
                        s = s_tiles.pop(g - 2)
                        e1 = work.tile([128, 2, NQ], F16, name="e1", tag="e1")
                        nc.scalar.activation(e1, s, AF.Exp)
                        et = work.tile([128, 2, NQ], F16, name="et", tag="et")
                        nc.vector.tensor_mul(et, e1, _bcast2(eB[c][:, :], 2))
                        nc.tensor.matmul(
                            o2a, Vo[c][:, h0, :], et[:, 0, :],
                            start=(c == 0), stop=(c == KC - 1))
                        nc.tensor.matmul(
                            o2b, Vo[c][:, h1, :], et[:, 1, :],
                            start=(c == 0), stop=(c == KC - 1))
                        fillers(hp, c)
                        if c == KC - 1:
                            # normalize: sums to SBUF, rank-2 selector
                            # matmul broadcasts them across the pair's
                            # partitions, fast approx reciprocal, one mul
                            oUp = norm.tile([128, NQ], F16, name="oUp", tag="oUp")
                            nc.vector.tensor_copy(oUp[0:DH, :], o2a[0:DH, :])
                            nc.vector.tensor_copy(oUp[DH:128, :], o2b[0:DH, :])
                            nc.vector.tensor_copy(ss2[0:1, :], o2a[DH:DH + 1, :])
                            nc.vector.tensor_copy(ss2[32:33, :], o2b[DH:DH + 1, :])
                            rb = psA.tile([128, NQ], F32, name="rb", tag="proj")
                            nc.tensor.matmul(rb, e2, ss2, start=True, stop=True)
                            rrb = norm.tile([128, NQ], F32, name="rrb", tag="rrb")
                            nc.vector.reciprocal_approx_fast(out=rrb, in_=rb)
                            nc.vector.tensor_mul(OT[hp], oUp, rrb)

            # ---- output projection + bias (ki-outer: the ki<3 partial
            # sums run while the last head pair is still normalizing;
            # bo enters as a rank-1 matmul opening the accumulation) ----
            with tc.tile_pool(name="psY", bufs=1, space="PSUM") as psY:
                pss = [psY.tile([128, NQ], F32, name=f"yTp{mi}", tag=f"yTp{mi}")
                       for mi in range(4)]
                for mi in range(4):
                    msl = slice(mi * 128, (mi + 1) * 128)
                    nc.tensor.matmul(
                        pss[mi], boro[:, msl], onesq,
                        start=True, stop=False)
                for ki in range(4):
                    for mi in range(4):
                        msl = slice(mi * 128, (mi + 1) * 128)
                        nc.tensor.matmul(
                            pss[mi], wo[ki][:, msl], OT[ki],
                            start=False, stop=(ki == 3))
                for mi in range(4):
                    msl = slice(mi * 128, (mi + 1) * 128)
                    ysb = norm.tile([128, NQ], F16, name="ysb", tag="ysb")
                    nc.scalar.copy(ysb, pss[mi])
                    if mi % 2 == 0:
                        nc.sync.dma_start(out=yT_d[msl, :], in_=ysb)
                    else:
                        nc.gpsimd.dma_start(out=yT_d[msl, :], in_=ysb)

    nc.compile()
    return nc


_NC_CACHE = {}


def _get_nc():
    if "nc" not in _NC_CACHE:
        _NC_CACHE["nc"] = _build_nc()
    return _NC_CACHE["nc"]


def make_in_maps(x, context, bias, Wq, Wk, Wv, Wo, bo):
    x = np.asarray(x, dtype=np.float32)
    context = np.asarray(context, dtype=np.float32)
    bias = np.asarray(bias, dtype=np.float32)
    wqT = np.ascontiguousarray((np.asarray(Wq) * SCALE).T).astype(np.float16)
    wkT = np.ascontiguousarray(np.asarray(Wk).T).astype(np.float16)
    wvT = np.ascontiguousarray(np.asarray(Wv).T).astype(np.float16)
    woT = np.ascontiguousarray(np.asarray(Wo).T).astype(np.float16)
    bo2 = np.ascontiguousarray(np.asarray(bo).reshape(1, D)).astype(np.float16)

    in_maps = []
    for core in range(8):
        b, half = core // 2, core % 2
        qs = half * NQ
        in_maps.append({
            "xT": np.ascontiguousarray(x[b, qs:qs + NQ, :].T).astype(np.float16),
            "ctxT": np.ascontiguousarray(context[b].T).astype(np.float16),
            "expB": np.ascontiguousarray(
                np.exp(bias[b, qs:qs + NQ, :] - BSHIFT).T).astype(np.float16),
            "wqT": wqT, "wkT": wkT, "wvT": wvT, "woT": woT, "bo": bo2,
        })
    return in_maps


def kernel(x, context, bias, Wq, Wk, Wv, Wo, bo):
    nc = _get_nc()
    in_maps = make_in_maps(x, context, bias, Wq, Wk, Wv, Wo, bo)
    res = bass_utils.run_bass_kernel_spmd(
        nc, in_maps, core_ids=list(range(8)), trace=False)

    out = np.empty((4, 2 * NQ, D), dtype=np.float32)
    for core in range(8):
        b, half = core // 2, core % 2
        qs = half * NQ
        out[b, qs:qs + NQ, :] = res.results[core]["yT"].astype(np.float32).T
    return out


# revision 21
# speedup vs baseline: 1.2992x; 1.0235x over previous
"""Cross-attention kernel for Trainium2 (8 NeuronCores, SPMD).

Problem: B=4, Nq=1024, Nk=2048, D=512, 8 heads x 64 head-dim, fp32,
full-tensor bias added to scores before softmax.

Sharding: (batch, head-half) -> 8 shards, one per core. Each core computes
4 heads over the full 1024 queries of its batch and emits a PARTIAL output
projection (its 256 inner dims of Wo); the host adds the two partials per
batch. This halves the K/V projection work per core versus query-sharding
(K/V no longer computed redundantly) at the cost of a fp16 partial-sum
gather on the host.

Device layout: attention tensors kept transposed (feature/key dim on
partitions) so every matmul contraction lands on the partition axis:
  QT[d, q] = (SCALE*Wq_hh) @ xT       KT[d, k] = Wk_hh @ ctxT
  V[k, i]  = ctxT.T @ Wv_hh.T
  ST[k, q] = KT_h.T @ QT_h            (two heads of a pair in PE row groups
                                       0-1/2-3, concurrent)
  E = exp(ST) * exp(biasT - 4)        (ACT exp; DVE multiply against a
                                       step-0 broadcast of the host-side
                                       exp(bias-4).T tile)
  out2T[i(+1), q] = [V_h | 1].T @ E   (ones column gives softmax row-sums)
  OT = out2T[0:64] * recip(sum)       (DVE sums->SBUF, rank-2 selector
                                       matmul broadcasts per-query factors
                                       across the pair's partitions, fast
                                       approx reciprocal, one multiply)
  yT_part[d, q] = Wo_hh @ OT + bo/2   (bo enters as a rank-1 matmul; ACT
                                       evacuates fp16 for the store)
The inner loop runs 64 units (pair, q-half, chunk) software-pipelined two
ahead (scores lead exp/mul/AV), with K/Q prefetch for the next pair and
the V stream as TensorE fillers during the first block.
"""

import numpy as np
import concourse.bass as bass
import concourse.bacc as bacc
import concourse.mybir as mybir
import concourse.tile as tile
from concourse import bass_utils

HEADS = 8
HPC = 4           # heads per core
DH = 64
D = 512
IN2 = HPC * DH    # 256 inner dims per core
NQ = 1024         # full queries per core
QH = 512          # query half (matmul moving width)
NK = 2048
KC = NK // 128    # 16 key chunks
SCALE = DH ** -0.5
BSHIFT = 4.0

F32 = mybir.dt.float32
F16 = mybir.dt.float16
AF = mybir.ActivationFunctionType


def _bcast2(ap, n):
    """[128, F] -> [128, n, F] with a step-0 middle dim."""
    return bass.AP(ap.tensor, ap.offset, [ap.ap[0], [0, n], ap.ap[1]])


def _build_nc():
    nc = bacc.Bacc("TRN2", target_bir_lowering=False, debug=False)

    xT_d = nc.dram_tensor("xT", [D, NQ], F16, kind="ExternalInput")
    ctxT_d = nc.dram_tensor("ctxT", [D, NK], F16, kind="ExternalInput")
    expB_d = nc.dram_tensor("expB", [NK, NQ], F16, kind="ExternalInput")
    wqT_d = nc.dram_tensor("wqT", [D, IN2], F16, kind="ExternalInput")
    wkT_d = nc.dram_tensor("wkT", [D, IN2], F16, kind="ExternalInput")
    wvT_d = nc.dram_tensor("wvT", [D, IN2], F16, kind="ExternalInput")
    woT_d = nc.dram_tensor("woT", [IN2, D], F16, kind="ExternalInput")
    bo_d = nc.dram_tensor("bo", [1, D], F16, kind="ExternalInput")
    yT_d = nc.dram_tensor("yT", [D, NQ], F16, kind="ExternalOutput")

    with tile.TileContext(nc) as tc, nc.allow_low_precision(
            reason="fp16 matmul operands, fp32 accumulation"):
        with (
            tc.tile_pool(name="const", bufs=1) as const,
            tc.tile_pool(name="main", bufs=1) as main,
            tc.tile_pool(name="work", bufs=6) as work,
            tc.tile_pool(name="norm", bufs=3) as norm,
            tc.tile_pool(name="ctxp", bufs=1) as ctxp,
        ):
            wq = [const.tile([128, IN2], F16, name=f"wq{i}", tag=f"wq{i}") for i in range(4)]
            wk = [const.tile([128, IN2], F16, name=f"wk{i}", tag=f"wk{i}") for i in range(4)]
            wv = [const.tile([128, IN2], F16, name=f"wv{i}", tag=f"wv{i}") for i in range(4)]
            wo = [const.tile([128, D], F16, name=f"wo{i}", tag=f"wo{i}") for i in range(2)]
            boro = const.tile([1, D], F16, name="boro", tag="boro")
            onesF = const.tile([128, 1], F32, name="onesF", tag="onesF")
            nc.vector.memset(onesF, 1.0)
            onesq = const.tile([1, NQ], F16, name="onesq", tag="onesq")
            nc.vector.tensor_copy(onesq, onesF[0:1, 0:1].broadcast_to([1, NQ]))
            e2 = const.tile([33, 128], F16, name="e2", tag="e2")
            nc.vector.memset(e2, 0.0)
            nc.vector.memset(e2[0:1, 0:DH], 1.0)
            nc.vector.memset(e2[32:33, DH:128], 1.0)
            ss2 = const.tile([33, QH], F16, name="ss2", tag="ss2")
            nc.vector.memset(ss2, 0.0)

            ctx = [ctxp.tile([128, NK], F16, name=f"ctx{i}", tag=f"ctx{i}") for i in range(4)]
            xts = [ctxp.tile([128, NQ], F16, name=f"xts{i}", tag=f"xts{i}") for i in range(4)]
            # sync queue: wk, full-tile ctx (big DMAs sustain the best
            # rate), then the exp(bias) stream, then tail-only weights
            for i in range(4):
                nc.sync.dma_start(out=wk[i], in_=wkT_d[i * 128:(i + 1) * 128, :])
            for i in range(4):
                nc.sync.dma_start(out=ctx[i], in_=ctxT_d[i * 128:(i + 1) * 128, :])
            # gpsimd queue: x + Wq for the early Q proj, then Wv
            for i in range(4):
                nc.gpsimd.dma_start(out=xts[i], in_=xT_d[i * 128:(i + 1) * 128, :])
            for i in range(4):
                nc.gpsimd.dma_start(out=wq[i], in_=wqT_d[i * 128:(i + 1) * 128, :])
            for i in range(4):
                nc.gpsimd.dma_start(out=wv[i], in_=wvT_d[i * 128:(i + 1) * 128, :])

            KT = [main.tile([128, NK], F16, name=f"KT{i}", tag=f"KT{i}") for i in range(2)]
            QT = [main.tile([128, NQ], F16, name=f"QT{i}", tag=f"QT{i}") for i in range(2)]
            OT = [main.tile([128, NQ], F16, name=f"OT{i}", tag=f"OT{i}") for i in range(2)]
            Vo = [main.tile([128, HPC, DH + 1], F16, name=f"Vo{c}", tag=f"Vo{c}")
                  for c in range(KC)]
            eB = [main.tile([128, NQ], F16, name=f"eB{c}", tag=f"eB{c}") for c in range(KC)]
            for c in range(KC):
                nc.vector.tensor_copy(
                    Vo[c][:, :, DH], onesF[:, 0:1].broadcast_to([128, HPC]))
            for c in range(KC):
                nc.sync.dma_start(out=eB[c], in_=expB_d[c * 128:(c + 1) * 128, :])
            for i in range(2):
                nc.sync.dma_start(out=wo[i], in_=woT_d[i * 128:(i + 1) * 128, :])
            nc.sync.dma_start(out=boro, in_=bo_d[:, :])

            def k_proj_group(psA, mi, nt):
                msl = slice(mi * 128, (mi + 1) * 128)
                nsl = slice(nt * 512, (nt + 1) * 512)
                ps = psA.tile([128, 512], F32, name="proj", tag="proj")
                for ki in range(4):
                    nc.tensor.matmul(
                        ps, wk[ki][:, msl], ctx[ki][:, nsl],
                        start=(ki == 0), stop=(ki == 3))
                nc.vector.tensor_copy(KT[mi][:, nsl], ps)

            def v_proj_group(psA, c, on_act=False):
                csl = slice(c * 128, (c + 1) * 128)
                ps = psA.tile([128, IN2], F32, name="vproj", tag="proj")
                for ki in range(4):
                    nc.tensor.matmul(
                        ps, ctx[ki][:, csl], wv[ki],
                        start=(ki == 0), stop=(ki == 3))
                src = ps.rearrange("p (h d) -> p h d", h=HPC)
                if on_act:
                    nc.scalar.copy(Vo[c][:, :, 0:DH], src)
                else:
                    nc.vector.tensor_copy(Vo[c][:, :, 0:DH], src)

            def q_proj_group(psA, mi):
                msl = slice(mi * 128, (mi + 1) * 128)
                for qh in range(2):
                    qsl = slice(qh * QH, (qh + 1) * QH)
                    ps = psA.tile([128, QH], F32, name="qproj", tag="proj")
                    for ki in range(4):
                        nc.tensor.matmul(
                            ps, wq[ki][:, msl], xts[ki][:, qsl],
                            start=(ki == 0), stop=(ki == 3))
                    nc.vector.tensor_copy(QT[mi][:, qsl], ps)

            # ---- upfront: just enough to start block (0,0) ----
            with tc.tile_pool(name="psA0", bufs=2, space="PSUM") as psA0:
                k_proj_group(psA0, 0, 0)
                k_proj_group(psA0, 0, 1)
                q_proj_group(psA0, 0)
                for c in range(2):
                    v_proj_group(psA0, c)

            # ---- attention: 64 units (pair, q-half, chunk), scores two
            # units ahead of the exp/mul/AV stage ----
            with (
                tc.tile_pool(name="psS", bufs=2, space="PSUM") as psS,
                tc.tile_pool(name="psO", bufs=2, space="PSUM") as psO,
                tc.tile_pool(name="psA", bufs=2, space="PSUM") as psA,
            ):
                lo, hi = slice(0, DH), slice(DH, 128)

                def fillers(hp, qh, c):
                    if hp == 0 and qh == 0:
                        if c == 0:
                            k_proj_group(psA, 0, 2)
                        elif c == 1:
                            k_proj_group(psA, 0, 3)
                        if c <= 13:
                            v_proj_group(psA, c + 2, on_act=(c % 3 == 2))
                        if c == 6:
                            k_proj_group(psA, 1, 0)
                        elif c == 10:
                            k_proj_group(psA, 1, 1)
                    elif hp == 0 and qh == 1:
                        if c == 0:
                            k_proj_group(psA, 1, 2)
                        elif c == 4:
                            k_proj_group(psA, 1, 3)
                        elif c == 8:
                            q_proj_group(psA, 1)

                def unit(g):
                    hp, r = divmod(g, 2 * KC)
                    qh, c = divmod(r, KC)
                    return hp, qh, c

                s_tiles, o2t = {}, {}
                for g in range(64 + 2):
                    if g < 64:
                        hp, qh, c = unit(g)
                        qsl = slice(qh * QH, (qh + 1) * QH)
                        csl = slice(c * 128, (c + 1) * 128)
                        s = psS.tile([128, 2, QH], F32, name="s", tag="s")
                        s_tiles[g] = s
                        nc.tensor.matmul(
                            s[:, 0, :], KT[hp][lo, csl], QT[hp][lo, qsl],
                            start=True, stop=True)
                        nc.tensor.matmul(
                            s[:, 1, :], KT[hp][hi, csl], QT[hp][hi, qsl],
                            start=True, stop=True)
                    if g >= 2:
                        hp, qh, c = unit(g - 2)
                        qsl = slice(qh * QH, (qh + 1) * QH)
                        h0, h1 = 2 * hp, 2 * hp + 1
                        if c == 0:
                            o2t[(hp, qh)] = (
                                psO.tile([DH + 1, QH], F32, name="o2a", tag="o2"),
                                psO.tile([DH + 1, QH], F32, name="o2b", tag="o2"))
                        o2a, o2b = o2t[(hp, qh)]
                        s = s_tiles.pop(g - 2)
                        e1 = work.tile([128, 2, QH], F16, name="e1", tag="e1")
                        nc.scalar.activation(e1, s, AF.Exp)
                        et = work.tile([128, 2, QH], F16, name="et", tag="et")
                        nc.vector.tensor_mul(et, e1, _bcast2(eB[c][:, qsl], 2))
                        nc.tensor.matmul(
                            o2a, Vo[c][:, h0, :], et[:, 0, :],
                            start=(c == 0), stop=(c == KC - 1))
                        nc.tensor.matmul(
                            o2b, Vo[c][:, h1, :], et[:, 1, :],
                            start=(c == 0), stop=(c == KC - 1))
                        fillers(hp, qh, c)
                        if c == KC - 1:
                            oUp = norm.tile([128, QH], F16, name="oUp", tag="oUp")
                            nc.vector.tensor_copy(oUp[0:DH, :], o2a[0:DH, :])
                            nc.vector.tensor_copy(oUp[DH:128, :], o2b[0:DH, :])
                            nc.vector.tensor_copy(ss2[0:1, :], o2a[DH:DH + 1, :])
                            nc.vector.tensor_copy(ss2[32:33, :], o2b[DH:DH + 1, :])
                            rb = psA.tile([128, QH], F32, name="rb", tag="proj")
                            nc.tensor.matmul(rb, e2, ss2, start=True, stop=True)
                            rrb = norm.tile([128, QH], F32, name="rrb", tag="rrb")
                            nc.vector.reciprocal_approx_fast(out=rrb, in_=rb)
                            nc.vector.tensor_mul(OT[hp][:, qsl], oUp, rrb)

            # ---- partial output projection + bias (half of bo per core;
            # the host adds the two partials per batch) ----
            with tc.tile_pool(name="psY", bufs=1, space="PSUM") as psY:
                pss = [psY.tile([128, NQ], F32, name=f"yTp{mi}", tag=f"yTp{mi}")
                       for mi in range(4)]
                for mi in range(4):
                    msl = slice(mi * 128, (mi + 1) * 128)
                    for qh in range(2):
                        qsl = slice(qh * QH, (qh + 1) * QH)
                        nc.tensor.matmul(
                            pss[mi][:, qsl], boro[:, msl], onesq[:, qsl],
                            start=True, stop=False)
                for ki in range(2):
                    for mi in range(4):
                        msl = slice(mi * 128, (mi + 1) * 128)
                        for qh in range(2):
                            qsl = slice(qh * QH, (qh + 1) * QH)
                            nc.tensor.matmul(
                                pss[mi][:, qsl], wo[ki][:, msl], OT[ki][:, qsl],
                                start=False, stop=(ki == 1))
                for mi in range(4):
                    msl = slice(mi * 128, (mi + 1) * 128)
                    ysb = norm.tile([128, NQ], F16, name="ysb", tag="ysb")
                    nc.scalar.copy(ysb, pss[mi])
                    if mi % 2 == 0:
                        nc.sync.dma_start(out=yT_d[msl, :], in_=ysb)
                    else:
                        nc.gpsimd.dma_start(out=yT_d[msl, :], in_=ysb)

    nc.compile()
    return nc


_NC_CACHE = {}


def _get_nc():
    if "nc" not in _NC_CACHE:
        _NC_CACHE["nc"] = _build_nc()
    return _NC_CACHE["nc"]


def make_in_maps(x, context, bias, Wq, Wk, Wv, Wo, bo):
    x = np.asarray(x, dtype=np.float32)
    context = np.asarray(context, dtype=np.float32)
    bias = np.asarray(bias, dtype=np.float32)
    Wq = np.asarray(Wq); Wk = np.asarray(Wk); Wv = np.asarray(Wv)
    Wo = np.asarray(Wo)
    # half of bo on each core so the host-side partial add reconstructs it
    bo2 = np.ascontiguousarray(
        (np.asarray(bo) * 0.5).reshape(1, D)).astype(np.float16)

    in_maps = []
    for core in range(8):
        b, hh = core // 2, core % 2
        hsl = slice(hh * IN2, (hh + 1) * IN2)
        in_maps.append({
            "xT": np.ascontiguousarray(x[b].T).astype(np.float16),
            "ctxT": np.ascontiguousarray(context[b].T).astype(np.float16),
            "expB": np.ascontiguousarray(
                np.exp(bias[b] - BSHIFT).T).astype(np.float16),
            "wqT": np.ascontiguousarray((Wq[hsl] * SCALE).T).astype(np.float16),
            "wkT": np.ascontiguousarray(Wk[hsl].T).astype(np.float16),
            "wvT": np.ascontiguousarray(Wv[hsl].T).astype(np.float16),
            "woT": np.ascontiguousarray(Wo[:, hsl].T).astype(np.float16),
            "bo": bo2,
        })
    return in_maps


def kernel(x, context, bias, Wq, Wk, Wv, Wo, bo):
    nc = _get_nc()
    in_maps = make_in_maps(x, context, bias, Wq, Wk, Wv, Wo, bo)
    res = bass_utils.run_bass_kernel_spmd(
        nc, in_maps, core_ids=list(range(8)), trace=False)

    out = np.empty((4, NQ, D), dtype=np.float32)
    for b in range(4):
        pa = res.results[2 * b]["yT"].astype(np.float32)
        pb = res.results[2 * b + 1]["yT"].astype(np.float32)
        out[b] = (pa + pb).T
    return out
